# revision 84
# baseline (speedup 1.0000x reference)
"""Multi-head self-attention (B=8, S=1024, D=768, H=12) on 8 trn2 cores.

Sharding: data-parallel over batch -- core b computes attention for Q[b].
No collectives.

Design notes (cost-model-driven):
- matmul cost = out free size x 0.4167ns (bf16); ACT cost = free size x
  0.8333ns + ~185ns; engines execute their stream IN ORDER; the PE
  p-state resets on idle gaps; HWDGE descgen is serial (~630ns/DMA);
  DMA_ENGINES is one exclusive device; <512B descriptors pay 2x.
- ctx is computed UNtransposed: out[s_q(128part), 65] = exp_slice.T @
  [v|1]; col 64 is Z=sum(exp) per PARTITION, so softmax normalization is
  a per-partition tensor_scalar (no partition broadcast), and ctx matmul
  cost is halved vs the transposed form.
- exp (ACT) is the bottleneck engine (~100us busy). The schedule is a
  lockstep: 48 score half-units (2 psum tiles + 2 exps each) paced by
  ACT, with all other PE work (projection chains, v, ctx, output) drawn
  from a filler queue between units to keep the PE from racing ahead or
  falling behind.
- host permutes qt columns (pos = sc*128+p <-> s = p*8+sc) so each
  head's output rows are partition-contiguous; output is [H, S, DK] and
  the host re-interleaves heads.
"""

import ml_dtypes
import numpy as np

import concourse.bass as bass
import concourse.mybir as mybir
import concourse.tile as tile
from concourse.bass_utils import run_bass_kernel_spmd

F32 = mybir.dt.float32
BF16 = mybir.dt.bfloat16

S = 1024
D = 768
H = 12
DK = 64
KC = D // 128   # 6 contraction chunks
MC = D // 128   # 6 W-row chunks (head pairs)
SC = S // 128   # 8 sequence chunks
NSQ = S // 512  # 2 query-column chunks
SCALE = 1.0 / np.sqrt(DK)
VROW = 65       # 64 v columns + 1 ones column per head
N_WARM = 7      # PE warmup matmuls (p-state ramp while input DMAs fly)


def _split_excess_waits(nc, max_waits=1):
    """This container's walrus encodes at most one sem-wait per
    instruction; spread extra waits onto EventSemaphore instructions."""
    for fn in nc.m.functions:
        for bb in fn.blocks:
            out = []
            for ins in bb.instructions:
                si = getattr(ins, "sync_info", None)
                ow = list(si.on_wait) if (si is not None and si.on_wait) else []
                if len(ow) > max_waits:
                    head, tail = ow[:-max_waits], ow[-max_waits:]
                    for j in range(0, len(head), max_waits):
                        ev = mybir.InstEventSemaphore(
                            name=f"evsplit-{ins.name}-{j}", ins=[], outs=[])
                        ev.engine = ins.engine
                        ev.sync_info = mybir.SyncInfo(
                            on_wait=head[j:j + max_waits], on_update=[])
                        out.append(ev)
                    ins.sync_info = mybir.SyncInfo(
                        on_wait=tail, on_update=list(si.on_update))
                out.append(ins)
            bb.instructions = out


def build_nc():
    nc = bass.Bass(trn_type="TRN2")

    # wqk0: W-row chunks 0-1 as [q c0 | k c0 | q c1 | k c1] so the first
    # 256 cols are exactly what pair-0 chains need; wqk1: chunks 2-5 of q
    # then of k. All host-fused to keep the DMA count low.
    qt = nc.dram_tensor("qt", [D, S], BF16, kind="ExternalInput").ap()
    wqk0 = nc.dram_tensor("wqk0", [D, 512], BF16, kind="ExternalInput").ap()
    wqk1 = nc.dram_tensor("wqk1", [D, 2 * (D - 256)], BF16,
                          kind="ExternalInput").ap()
    wvt = nc.dram_tensor("wvt", [D, D], BF16, kind="ExternalInput").ap()
    bqk = nc.dram_tensor("bqk", [2, D], F32, kind="ExternalInput").ap()
    bvb = nc.dram_tensor("bvb", [128, D], BF16, kind="ExternalInput").ap()
    y = nc.dram_tensor("y", [H, S, DK], F32, kind="ExternalOutput").ap()

    with tile.TileContext(nc) as tc:
        with (
            tc.tile_pool(name="singles", bufs=1) as singles,
            tc.tile_pool(name="psP", bufs=2, space="PSUM") as psP,
            tc.tile_pool(name="psA", bufs=2, space="PSUM") as psA,
            tc.tile_pool(name="psC", bufs=2, space="PSUM") as psC,
            tc.tile_pool(name="expp", bufs=48) as expp,
            tc.tile_pool(name="recp", bufs=8) as recp,
        ):
            # ---- persistent SBUF arrays --------------------------------
            wrm = singles.tile([128, 512], BF16)          # warmup zeros
            qt_sb = singles.tile([128, KC, S], BF16)      # X^T
            wqk0_sb = singles.tile([128, KC, 512], BF16)  # W chunks 0-1
            wqk1_sb = singles.tile([128, KC, 2 * (D - 256)], BF16)
            wv_sb = singles.tile([128, KC, D], BF16)
            qT_sb = singles.tile([128, MC, S], BF16)      # q^T (d_out part)
            kT_sb = singles.tile([128, MC, S], BF16)
            v_sb = singles.tile([128, SC, H * VROW], BF16)
            bqk_sb = singles.tile([128, 2, MC], F32)
            bvb_sb = singles.tile([128, H, DK], BF16)
            y_sb = [singles.tile([128, SC, DK], F32, name=f"y{h}")
                    for h in range(H)]

            v4 = v_sb.rearrange("p s (h c) -> p s h c", c=VROW)

            # ---- warmup: keep the PE busy (p-state ramp) while inputs
            # load. Zeros in, zeros out, results never read.
            nc.vector.memset(wrm, 0.0)
            for i in range(N_WARM):
                ps = psC.tile([128, 512], F32, tag="ctx", name=f"warm_{i}")
                nc.tensor.matmul(ps, lhsT=wrm[:, 0:128], rhs=wrm,
                                 start=True, stop=True)
            nc.vector.memset(v4[:, :, :, DK:DK + 1], 1.0)  # ones columns

            # ---- input DMAs: one queue, strict priority order ----------
            qtr = qt.rearrange("(c p) s -> p c s", p=128)
            wqk0r = wqk0.rearrange("(c p) n -> p c n", p=128)
            nc.sync.dma_start(out=wqk0_sb[:, :, 0:256], in_=wqk0r[:, :, 0:256])
            nc.sync.dma_start(out=qt_sb[:, 0:3, 0:512], in_=qtr[:, 0:3, 0:512])
            nc.sync.dma_start(out=bqk_sb,
                              in_=bqk.rearrange("t (c p) -> p t c", p=128))
            nc.sync.dma_start(out=qt_sb[:, 3:6, 0:512], in_=qtr[:, 3:6, 0:512])
            nc.sync.dma_start(out=wqk0_sb[:, :, 256:512],
                              in_=wqk0r[:, :, 256:512])
            nc.sync.dma_start(out=qt_sb[:, :, 512:1024],
                              in_=qtr[:, :, 512:1024])
            nc.sync.dma_start(out=wqk1_sb,
                              in_=wqk1.rearrange("(c p) n -> p c n", p=128))
            nc.sync.dma_start(out=wv_sb,
                              in_=wvt.rearrange("(c p) n -> p c n", p=128))
            nc.sync.dma_start(out=bvb_sb,
                              in_=bvb.rearrange("p (h c) -> p h c", c=DK))

            # ---- emission helpers --------------------------------------

            def wslice(t, c):
                """t=0 -> Wq rows, t=1 -> Wk rows, for row chunk c."""
                if c < 2:
                    o = (2 * c + t) * 128
                    return wqk0_sb[:, :, o:o + 128]
                o = t * (D - 256) + (c - 2) * 128
                return wqk1_sb[:, :, o:o + 128]

            def chain(t, c, n):
                """One projection chain: W-rows chunk c (q if t=0, k if
                t=1) x qt cols n*512:+512 -> qT/kT slice + bias."""
                o_sb = (qT_sb, kT_sb)[t]
                ps = psP.tile([128, 512], F32, tag="proj",
                              name=f"pj_{t}_{c}_{n}")
                for kc in range(KC):
                    nc.tensor.matmul(
                        ps,
                        lhsT=wslice(t, c)[:, kc, :],
                        rhs=qt_sb[:, kc, n * 512:(n + 1) * 512],
                        start=(kc == 0), stop=(kc == KC - 1),
                    )
                nc.vector.tensor_scalar_add(
                    out=o_sb[:, c, n * 512:(n + 1) * 512],
                    in0=ps,
                    scalar1=bqk_sb[:, t, c:c + 1],
                )

            def proj_v(sc, n):
                """One v chunk: v[sc*128:+128, n*384:+384] = X@Wv^T + bv."""
                ps = psP.tile([128, 512], F32, tag="proj",
                              name=f"pv_{sc}_{n}")
                for kc in range(KC):
                    nc.tensor.matmul(
                        ps[:, 0:384],
                        lhsT=qt_sb[:, kc, sc * 128:(sc + 1) * 128],
                        rhs=wv_sb[:, kc, n * 384:(n + 1) * 384],
                        start=(kc == 0), stop=(kc == KC - 1),
                    )
                nc.vector.tensor_add(
                    out=v4[:, sc, 6 * n:6 * n + 6, 0:DK],
                    in0=ps[:, 0:384].rearrange("p (h c) -> p h c", c=DK),
                    in1=bvb_sb[:, 6 * n:6 * n + 6, :],
                )

            exps = {}

            def sc_exp(h, j, half):
                """scores^T + exp for head h, query cols j*512:+512, s_k
                chunks 4*half..+4: two [128,2,512] psums, two ACT exps."""
                mc, pb = h // 2, (h % 2) * DK
                for g in (2 * half, 2 * half + 1):
                    ps = psA.tile([128, 2, 512], F32, tag="sc",
                                  name=f"sc_{h}_{j}_{g}")
                    for i in range(2):
                        kc2 = 2 * g + i
                        nc.tensor.matmul(
                            ps[:, i, :],
                            lhsT=kT_sb[pb:pb + DK, mc,
                                       kc2 * 128:(kc2 + 1) * 128],
                            rhs=qT_sb[pb:pb + DK, mc,
                                      j * 512:(j + 1) * 512],
                            start=True, stop=True,
                        )
                    exp_t = expp.tile([128, 2, 512], BF16, tag="exp",
                                      name=f"exp_{h}_{j}_{g}")
                    nc.scalar.activation(
                        out=exp_t,
                        in_=ps,
                        func=mybir.ActivationFunctionType.Exp,
                        scale=float(SCALE),
                    )
                    exps.setdefault((h, j), []).append(exp_t)

            yr = y.rearrange("h (p c) d -> h p c d", c=SC)

            def ctx(h, j):
                """Untransposed ctx for head h, query rows j*512:+512.
                out[s_q(128), 0:64] = sum_k exp * v; col 64 = Z. Two chunk
                chains run g-interleaved so each exp group is consumed as
                it lands."""
                exp_ts = exps.pop((h, j))
                for half in range(2):
                    cs = (2 * half, 2 * half + 1)
                    pss = [psC.tile([128, 512], F32, tag="ctx",
                                    name=f"ct_{h}_{j}_{c}") for c in cs]
                    for g in range(4):
                        for i, c in enumerate(cs):
                            for t in range(2):
                                kc2 = 2 * g + t
                                nc.tensor.matmul(
                                    pss[i][:, 0:VROW],
                                    lhsT=exp_ts[g][:, t,
                                                   c * 128:(c + 1) * 128],
                                    rhs=v_sb[:, kc2, h * VROW:(h + 1) * VROW],
                                    start=(kc2 == 0), stop=(kc2 == SC - 1),
                                )
                    for i, c in enumerate(cs):
                        sc = 4 * j + c
                        rec = recp.tile([128, 1], F32, tag="rec",
                                        name=f"rec_{h}_{j}_{c}")
                        nc.vector.reciprocal(out=rec, in_=pss[i][:, DK:DK + 1])
                        nc.vector.tensor_scalar_mul(
                            out=y_sb[h][:, sc, :],
                            in0=pss[i][:, 0:DK],
                            scalar1=rec,
                        )
                    nc.sync.dma_start(
                        out=yr[h, :, sc - 1:sc + 1, :],
                        in_=y_sb[h][:, sc - 1:sc + 1, :])

            # ---- lockstep schedule -------------------------------------
            # 48 score half-units paced by ACT (~2.08us each); PE filler
            # drawn between units with a carried budget. need_fi forces
            # dependency fillers (projection chains) before units that
            # read them.
            units = []      # (h, j, half)
            for mc in range(MC):
                for h in (2 * mc, 2 * mc + 1):
                    for j in range(NSQ):
                        for half in range(2):
                            units.append((h, j, half))
            uidx = {}
            for i, (h, j, half) in enumerate(units):
                if half == 1:
                    uidx[(h, j)] = i + 1

            fillers = []    # (PE ns, ready_after_unit, closure)
            chain_fi = {}   # (t, c, n) -> filler index

            def add_chain(t, c, n):
                chain_fi[(t, c, n)] = len(fillers)
                fillers.append((1280, 0, lambda t=t, c=c, n=n: chain(t, c, n)))

            add_chain(1, 0, 1)
            add_chain(0, 0, 1)
            for c in (1, 2):
                add_chain(0, c, 0)
                add_chain(1, c, 0)
                add_chain(1, c, 1)
                add_chain(0, c, 1)
            def add_ctx(h, j):
                fillers.append(
                    (870, uidx[(h, j)], lambda h=h, j=j: ctx(h, j)))

            for sc in range(SC):
                for n in range(2):
                    fillers.append((960, 0, lambda sc=sc, n=n: proj_v(sc, n)))
            for h in (0, 1):
                for j in range(NSQ):
                    add_ctx(h, j)
            for c in (3, 4, 5):
                add_chain(0, c, 0)
                add_chain(1, c, 0)
                add_chain(1, c, 1)
                add_chain(0, c, 1)
                for h in (2 * (c - 2), 2 * (c - 2) + 1):
                    for j in range(NSQ):
                        add_ctx(h, j)
            for h in range(8, 12):
                for j in range(NSQ):
                    add_ctx(h, j)

            def need_fi(h, j, half):
                mc = h // 2
                need = -1
                for key in ((0, mc, j), (1, mc, half)):
                    if key in chain_fi:
                        need = max(need, chain_fi[key])
                return need + 1

            # pair-0 opening chains (emitted inline, not fillers)
            chain(0, 0, 0)
            chain(1, 0, 0)

            # fillers go in a LOW priority band so the list scheduler never
            # prefers ready ctx/proj work over the score matmuls that feed
            # the bottleneck ACT engine.
            lowpri = [1_000_000]

            def low(emit):
                save = tc.cur_priority
                tc.cur_priority = lowpri[0]
                emit()
                lowpri[0] = tc.cur_priority
                tc.cur_priority = save

            # draw fillers paced evenly by cumulative PE cost so the need-
            # force never has to dump a big batch at once
            total_cost = sum(f[0] for f in fillers)
            fi = 0
            drawn = 0
            for u, (h, j, half) in enumerate(units):
                # force chains needed by this unit AND the unit after next
                # (the chain->bias->scores latency spans ~2 units)
                need = max(need_fi(h, j, half),
                           need_fi(*units[min(u + 2, len(units) - 1)]))
                lo = (lambda f: low(f)) if u < 36 else (lambda f: f())
                while fi < need:
                    drawn += fillers[fi][0]
                    lo(fillers[fi][2])
                    fi += 1
                tc.cur_wait_ts = 8600 + 2040 * u
                sc_exp(h, j, half)
                tc.cur_wait_ts = None
                if u >= len(units) - 6:
                    target = total_cost
                else:
                    target = total_cost * (u + 1.0) / len(units)
                while fi < len(fillers) and drawn < target:
                    cost, ready, emit = fillers[fi]
                    if ready > u + 1:
                        break
                    lo(emit)
                    drawn += cost
                    fi += 1
            while fi < len(fillers):
                low(fillers[fi][2])
                fi += 1

    _split_excess_waits(nc)
    return nc


_NC_CACHE = None


def _get_nc():
    global _NC_CACHE
    if _NC_CACHE is None:
        _NC_CACHE = build_nc()
    return _NC_CACHE


def kernel(Q, Wq, bq, Wk, bk, Wv, bv):
    BF = ml_dtypes.bfloat16
    Q = np.asarray(Q, np.float32)
    wqt = np.asarray(Wq, np.float32).T.astype(BF)
    wkt = np.asarray(Wk, np.float32).T.astype(BF)
    wqk0 = np.ascontiguousarray(np.concatenate(
        [wqt[:, 0:128], wkt[:, 0:128], wqt[:, 128:256], wkt[:, 128:256]],
        axis=1))
    wqk1 = np.ascontiguousarray(
        np.concatenate([wqt[:, 256:D], wkt[:, 256:D]], axis=1))
    wvt = np.ascontiguousarray(np.asarray(Wv, np.float32).T.astype(BF))
    bqk = np.ascontiguousarray(
        np.stack([np.asarray(bq, np.float32), np.asarray(bk, np.float32)]))
    bvb = np.ascontiguousarray(
        np.tile(np.asarray(bv, np.float32).astype(BF)[None, :], (128, 1)))
    # column permutation: device position sc*128 + p  <->  s = p*8 + sc
    pos = np.arange(S)
    perm = (pos % 128) * SC + pos // 128

    nc = _get_nc()
    in_maps = []
    for b in range(Q.shape[0]):
        qtb = Q[b].T.astype(BF)
        in_maps.append({
            "qt": np.ascontiguousarray(qtb[:, perm]),
            "wqk0": wqk0, "wqk1": wqk1, "wvt": wvt,
            "bqk": bqk, "bvb": bvb,
        })
    res = run_bass_kernel_spmd(nc, in_maps, core_ids=list(range(len(in_maps))))
    out = np.stack([
        np.asarray(r["y"]).transpose(1, 0, 2).reshape(S, D)
        for r in res.results])
    return np.ascontiguousarray(out)
